# revision 32
# baseline (speedup 1.0000x reference)
"""Multi-head self-attention on 8 Trainium2 NeuronCores.

Sharding: tensor-parallel over heads (2 heads per core, both batch elements
on every core). Each core computes qkv projection / attention / its slice of
the output projection (rows of W_out for its heads), producing a partial
[B, N, D] output; the host sums the 8 partials (bf16) and adds b_out.

Per-core dataflow (ACT exp is the pacing engine at ~1.1us per 512-q window;
everything else is scheduled to hide under it):
  - host supplies x^T [B, D, N] so the QKV projection can run directly
  - QKV^T = Wsel^T @ x^T -> Q^T, K^T, V^T, each [128=2*64 head rows, N],
    emitted in [128,512] units split into 2-matmul filler quanta
  - V^T is PE-transposed back to V [k, e] blocks with a ones column per head
    so the P@V matmul also produces the softmax row-sums for free
  - attention runs in (b, qh) chunks of QCH=512 q columns, 16 key blocks
    each; per window: two 64-row scores matmuls (dual-issued PE row groups)
    into one [128,1024] PSUM tile, ONE exp activation for both heads, the
    previous window's two PV matmuls, then ~450ns of filler quanta
  - PSUM banks: scores double-buffer 4 + PV accumulators 2 + filler pool 2
  - normalize: PV psum is evacuated immediately (row 64 = denominator via
    DMA to partition 0, rows 0..63 via vector copy) so the accumulator bank
    frees early; reciprocal_approx_fast + gpsimd partition_broadcast +
    multiply (head0 writes O^T in place; head1 DMA-shifts partitions)
  - y_partial = O^T-block^T @ W_out_slice, evacuated bf16
"""

import numpy as np
import ml_dtypes

B, N, D, H, DP = 2, 2048, 1024, 16, 64
SCALE = float(DP) ** 0.5
NCORES = 8
HC = H // NCORES            # heads per core = 2
E = HC * DP                 # per-core head-dim total = 128
QCH = 512                   # q columns per attention chunk
NQ = N // QCH               # 4 chunks per batch
KB = N // 128               # 16 k blocks
DC = D // 128               # 8 contraction chunks for the qkv projection
NU = N // 512               # 4 projection column units per eb

BF16 = ml_dtypes.bfloat16

_CACHE = {}

# rough PE issue costs (ns) used only for static filler pacing
MM_NS = 226
TR_NS = 120


def _build_bass(with_bias=False):
    import concourse.bass as bass
    import concourse.mybir as mybir
    import concourse.tile as tile
    from concourse import bacc
    from concourse.masks import make_identity

    MM_DT = mybir.dt.bfloat16    # matmul input dtype
    P_DT = mybir.dt.bfloat16     # exp(S^T) storage dtype
    F32 = mybir.dt.float32

    # nonzero b_qkv is handled by an extra contraction chunk whose x^T rows
    # are [ones, 0...] and whose weight rows carry the bias (bias as matmul)
    DCX = DC + (1 if with_bias else 0)
    VAW = 130  # VA free width: 2 heads x [V(64) | ones]
    nc = bacc.Bacc(None, target_bir_lowering=False)
    xt = nc.dram_tensor("xt", [B, DCX * 128, N], MM_DT, kind="ExternalInput")[:]
    wsel = nc.dram_tensor("wsel", [DCX * 128, 3 * E], MM_DT, kind="ExternalInput")[:]
    wout = nc.dram_tensor("wout", [E, D], MM_DT, kind="ExternalInput")[:]
    y = nc.dram_tensor("y", [B, N, D], MM_DT, kind="ExternalOutput")[:]

    with tile.TileContext(nc) as tc:
        with (
            tc.tile_pool(name="consts", bufs=1) as consts,
            tc.tile_pool(name="xtp", bufs=2) as xtp,
            tc.tile_pool(name="ptp", bufs=2) as ptp,
            tc.tile_pool(name="qkvp", bufs=2) as qkvp,
            tc.tile_pool(name="vap", bufs=2) as vap,
            tc.tile_pool(name="otp", bufs=2) as otp,
            tc.tile_pool(name="evacp", bufs=2) as evacp,
            tc.tile_pool(name="normp", bufs=2) as normp,
            # PSUM: scores dbuf 2x[128,1024]f32 = 4 banks; pv 2x[65,512]f32
            # = 2 banks; filler 2x[128,512]f32 = 2 banks -> 8 banks total
            tc.tile_pool(name="ps_sc", bufs=2, space="PSUM") as ps_sc,
            tc.tile_pool(name="ps_pv", bufs=2, space="PSUM") as ps_pv,
            tc.tile_pool(name="ps_fl", bufs=2, space="PSUM") as ps_fl,
        ):
            # ---- input DMAs in n-column chunks: the first chunk's worth of
            # x^T (all contraction rows, tokens 0..511) lands in ~2.5us so
            # the first projections can start immediately
            WS = consts.tile([128, DCX, 3 * E], MM_DT)
            wselr = wsel.rearrange("(dc p) e -> p dc e", p=128)
            nc.sync.dma_start(out=WS[:, 0 : DCX // 2, :], in_=wselr[:, 0 : DCX // 2, :])
            XTs = []
            for b in range(B):
                XT = xtp.tile([128, DCX, N], MM_DT, tag="xt", name="xt")
                XTs.append(XT)
            xtb0 = xt[0].rearrange("(dc p) n -> p dc n", p=128)
            xtb1 = xt[1].rearrange("(dc p) n -> p dc n", p=128)
            nc.sync.dma_start(
                out=XTs[0][:, 0 : DCX // 2, 0:512],
                in_=xtb0[:, 0 : DCX // 2, 0:512],
            )
            nc.sync.dma_start(
                out=WS[:, DCX // 2 : DCX, :], in_=wselr[:, DCX // 2 : DCX, :]
            )
            nc.sync.dma_start(
                out=XTs[0][:, DCX // 2 : DCX, 0:512],
                in_=xtb0[:, DCX // 2 : DCX, 0:512],
            )
            for u in range(1, NU):
                nc.sync.dma_start(
                    out=XTs[0][:, :, u * 512 : (u + 1) * 512],
                    in_=xtb0[:, :, u * 512 : (u + 1) * 512],
                )
            WOUT = consts.tile([128, D], MM_DT)
            nc.sync.dma_start(out=WOUT, in_=wout)
            for u in range(NU):
                nc.sync.dma_start(
                    out=XTs[1][:, :, u * 512 : (u + 1) * 512],
                    in_=xtb1[:, :, u * 512 : (u + 1) * 512],
                )

            IDENT = consts.tile([128, 128], MM_DT)
            make_identity(nc, IDENT)
            WARM = consts.tile([1, 1], F32)
            nc.vector.memset(WARM, 0.0)
            nc.scalar.activation(
                out=WARM, in_=WARM, func=mybir.ActivationFunctionType.Exp
            )
            QKVTs, VAs = [], []
            for b in range(B):
                QKVTs.append(
                    [
                        qkvp.tile([128, N], MM_DT, tag=f"qkv{eb}", name=f"qkv{eb}")
                        for eb in range(3)
                    ]
                )
                # V blocks with ones column per head: [V_h0(64)|1|V_h1(64)|1]
                VA = vap.tile([128, KB, VAW], MM_DT, tag="va", name="va")
                nc.gpsimd.memset(VA[:, :, DP : DP + 1], 1.0)
                nc.gpsimd.memset(VA[:, :, VAW // 2 + DP : VAW // 2 + DP + 1], 1.0)
                VAs.append(VA)

            # ---- unit emitters (split into small quanta for window pacing)
            def qkv_quanta(b2, eb, u):
                """QKV projection unit: 512 q cols of Q/K/V^T, DCX-chunk
                accumulation in one filler-psum bank; 1 matmul per quantum."""
                state = {}
                qs = []
                for dc in range(DCX):
                    def quant(dc=dc):
                        if dc == 0:
                            state["ps"] = ps_fl.tile(
                                [128, 512], F32, tag="fl", name="ps_qkv"
                            )
                        nc.tensor.matmul(
                            state["ps"],
                            lhsT=WS[:, dc, eb * 128 : (eb + 1) * 128],
                            rhs=XTs[b2][:, dc, u * 512 : (u + 1) * 512],
                            start=(dc == 0),
                            stop=(dc == DCX - 1),
                        )
                        if dc == DCX - 1:
                            nc.vector.tensor_copy(
                                out=QKVTs[b2][eb][:, u * 512 : (u + 1) * 512],
                                in_=state["ps"],
                            )
                    qs.append((MM_NS, quant))
                return qs

            def vtrans_quantum(b2, kc):
                def quant():
                    pst = ps_fl.tile([128, 128], MM_DT, tag="fl", name="ps_vt")
                    nc.tensor.transpose(
                        pst, QKVTs[b2][2][:, kc * 128 : (kc + 1) * 128], IDENT
                    )
                    nc.vector.tensor_copy(out=VAs[b2][:, kc, 0:DP], in_=pst[:, 0:DP])
                    nc.vector.tensor_copy(
                        out=VAs[b2][:, kc, VAW // 2 : VAW // 2 + DP],
                        in_=pst[:, DP : 2 * DP],
                    )
                return (TR_NS, quant)

            def proj_quanta(b2, OT2, nb, engines=("vector", "vector")):
                """Output projection for one 128-row block of y. The PSUM
                evacuation engine is selectable so the drain phase can spread
                casts over the idle Scalar engine."""
                state = {}
                qs = []
                for half in range(2):
                    def quant(half=half):
                        if half == 0:
                            state["ysb"] = evacp.tile(
                                [128, D], MM_DT, tag="y", name="ysb"
                            )
                        py = ps_fl.tile([128, 512], F32, tag="fl", name="ps_y")
                        nc.tensor.matmul(
                            py,
                            lhsT=OT2[:, nb * 128 : (nb + 1) * 128],
                            rhs=WOUT[:, half * 512 : (half + 1) * 512],
                            start=True,
                            stop=True,
                        )
                        dst = state["ysb"][:, half * 512 : (half + 1) * 512]
                        if engines[half] == "scalar":
                            nc.scalar.activation(
                                out=dst,
                                in_=py,
                                func=mybir.ActivationFunctionType.Copy,
                            )
                        else:
                            nc.vector.tensor_copy(out=dst, in_=py)
                        if half == 1:
                            nc.sync.dma_start(
                                out=y[b2, nb * 128 : (nb + 1) * 128, :],
                                in_=state["ysb"],
                            )
                    qs.append((MM_NS, quant))
                return qs

            # ---- pre-attention phase: just enough of b0's projections for
            # the first windows; everything else becomes paced filler work
            for eb in (1, 0):                         # b0: K u0, Q u0
                for c, q in qkv_quanta(0, eb, 0):
                    q()

            # prep_q: b0's remaining projections (ordered by the window that
            # first needs them) + b=1's first-chunk essentials. Must drain
            # before b=1's attention starts. prep2_q: rest of b=1's prep,
            # popped after prep_q (normally completes during b0 qh1-3).
            prep_q = qkv_quanta(0, 2, 0)                       # V u0
            prep_q += [vtrans_quantum(0, kc) for kc in range(4)]
            prep_q += qkv_quanta(0, 1, 1)                      # K u1
            prep_q += qkv_quanta(0, 2, 1)                      # V u1
            prep_q += [vtrans_quantum(0, kc) for kc in range(4, 8)]
            prep_q += qkv_quanta(0, 1, 2)                      # K u2
            prep_q += qkv_quanta(0, 2, 2)                      # V u2
            prep_q += [vtrans_quantum(0, kc) for kc in range(8, 12)]
            prep_q += qkv_quanta(0, 1, 3)                      # K u3
            prep_q += qkv_quanta(0, 2, 3)                      # V u3
            prep_q += [vtrans_quantum(0, kc) for kc in range(12, 16)]
            prep_q += qkv_quanta(0, 0, 1)                      # Q u1
            prep_q += qkv_quanta(0, 0, 2)                      # Q u2
            prep_q += qkv_quanta(0, 0, 3)                      # Q u3
            prep_q += qkv_quanta(1, 1, 0)                      # b1 K u0
            prep_q += qkv_quanta(1, 0, 0)                      # b1 Q u0
            prep_q += qkv_quanta(1, 2, 0)                      # b1 V u0
            prep_q += [vtrans_quantum(1, kc) for kc in range(4)]
            prep2_q = []
            for u in range(1, NU):
                prep2_q += qkv_quanta(1, 1, u)                 # b1 K u
            for u in range(1, NU):
                prep2_q += qkv_quanta(1, 2, u)                 # b1 V u
                prep2_q += [
                    vtrans_quantum(1, kc) for kc in range(4 * u, 4 * u + 4)
                ]
            for u in range(1, NU):
                prep2_q += qkv_quanta(1, 0, u)                 # b1 Q u

            # proj queue: (ready_window, cost, fn); popped when prep is empty
            proj_q = []
            gw = [0]  # global window counter

            def pop_fillers(budget):
                spent = 0
                while True:
                    if prep_q:
                        c, fn = prep_q[0]
                        src = prep_q
                    elif prep2_q:
                        c, fn = prep2_q[0]
                        src = prep2_q
                    elif proj_q and proj_q[0][0] <= gw[0]:
                        _, c, fn = proj_q[0]
                        src = proj_q
                    else:
                        return
                    if spent and spent + c > budget:
                        return
                    src.pop(0)
                    fn()
                    spent += c

            # ---- attention chunks (tails are software-pipelined: a chunk's
            # last two PV pairs + normalize are emitted inside the NEXT
            # chunk's second window so the exp stream never pauses at chunk
            # boundaries)
            chunks = [(b, qh) for b in range(B) for qh in range(NQ)]
            RING = 4
            pending_tail = [None]

            def norm_chain(pvs, OT, q0, c0, cw, copy_eng="vector"):
                """Normalize cw columns [c0, c0+cw) of this chunk's PV psum
                accumulators into O^T."""
                for h in range(HC):
                    pv = pvs[h]
                    oc = normp.tile([DP + 1, cw], F32, tag=f"oc{h}", name="oc")
                    if copy_eng == "scalar":
                        nc.scalar.activation(
                            out=oc,
                            in_=pv[:, c0 : c0 + cw],
                            func=mybir.ActivationFunctionType.Copy,
                        )
                    else:
                        nc.vector.tensor_copy(out=oc, in_=pv[:, c0 : c0 + cw])
                    rt = normp.tile([1, cw], F32, tag=f"rt{h}", name="rt")
                    nc.sync.dma_start(out=rt, in_=oc[DP : DP + 1, :])
                    ri = normp.tile([1, cw], F32, tag=f"ri{h}", name="ri")
                    nc.vector.reciprocal_approx_fast(out=ri, in_=rt)
                    bc = normp.tile([DP, cw], F32, tag=f"bc{h}", name="bc")
                    nc.gpsimd.partition_broadcast(bc, ri)
                    dst0 = q0 + c0
                    if h == 0:
                        nc.vector.tensor_mul(
                            out=OT[0:DP, dst0 : dst0 + cw],
                            in0=oc[0:DP, :],
                            in1=bc,
                        )
                    else:
                        ots = normp.tile([DP, cw], MM_DT, tag="ots", name="ots")
                        nc.vector.tensor_mul(out=ots, in0=oc[0:DP, :], in1=bc)
                        nc.sync.dma_start(
                            out=OT[DP : 2 * DP, dst0 : dst0 + cw], in_=ots
                        )

            for b, qh in chunks:
                if (b, qh) == (1, 0):
                    while prep_q:
                        prep_q.pop(0)[1]()
                QT, KT, VT = QKVTs[b]
                VA = VAs[b]
                if qh == 0:
                    OT = otp.tile([128, N], MM_DT, tag="ot", name="ot")
                    OTs_b = OT
                else:
                    OT = OTs_b
                q0 = qh * QCH
                PT = ptp.tile([128, RING, 2 * QCH], P_DT, tag="pt", name="pt")
                pvs = [
                    ps_pv.tile([DP + 1, QCH], F32, tag="pv", name=f"pv{h}")
                    for h in range(HC)
                ]

                def pv_mms(kc, pvs=pvs, VA=VA, PT=PT):
                    for h in range(HC):
                        nc.tensor.matmul(
                            pvs[h],
                            lhsT=VA[
                                :, kc, h * (VAW // 2) : h * (VAW // 2) + DP + 1
                            ],
                            rhs=PT[:, kc % RING, h * QCH : (h + 1) * QCH],
                            start=(kc == 0),
                            stop=(kc == KB - 1),
                        )

                budget = 1200 if (b, qh) == (0, 0) else (620 if b == 0 else 330)
                for kc in range(KB):
                    ssc = ps_sc.tile([128, 2 * QCH], F32, tag="sc", name="ssc")
                    for h in range(HC):
                        nc.tensor.matmul(
                            ssc[:, h * QCH : (h + 1) * QCH],
                            lhsT=KT[
                                h * DP : (h + 1) * DP,
                                kc * 128 : (kc + 1) * 128,
                            ],
                            rhs=QT[h * DP : (h + 1) * DP, q0 : q0 + QCH],
                            start=True,
                            stop=True,
                        )
                    nc.scalar.activation(
                        out=PT[:, kc % RING, :],
                        in_=ssc,
                        func=mybir.ActivationFunctionType.Exp,
                        scale=1.0 / SCALE,
                    )
                    if kc == 1 and pending_tail[0] is not None:
                        pending_tail[0]()
                        pending_tail[0] = None
                    # lag-2 PV so it never waits on the just-issued exp
                    if kc >= 2:
                        pv_mms(kc - 2)
                    pop_fillers(budget)
                    gw[0] += 1

                def tail(pvs=pvs, OT=OT, q0=q0, b=b, qh=qh, pv_mms=pv_mms):
                    pv_mms(KB - 2)
                    pv_mms(KB - 1)
                    norm_chain(pvs, OT, q0, 0, QCH)
                    # reserve the second-to-last chunk's last blocks for the
                    # drain phase (they fill the final normalize latency)
                    reserve = (b, qh) == (B - 1, NQ - 2)
                    nbs = range(qh * QCH // 128, (qh + 1) * QCH // 128)
                    for j, nb in enumerate(nbs):
                        if reserve and j >= 2:
                            for c, fn in proj_quanta(
                                b, OT, nb, engines=("vector", "scalar")
                            ):
                                proj_q.append((10**9, c, fn))
                        else:
                            for c, fn in proj_quanta(b, OT, nb):
                                proj_q.append((gw[0] + 5, c, fn))

                if (b, qh) != chunks[-1]:
                    pending_tail[0] = tail
                else:
                    # drain: piecewise normalize so the output projection
                    # pipelines with the chain; spread evacuation casts over
                    # the idle Scalar engine; reserved proj blocks fill the
                    # final normalize latency
                    pv_mms(KB - 2)
                    pv_mms(KB - 1)
                    while proj_q:
                        proj_q.pop(0)[2]()
                    engs = [("vector", "scalar"), ("scalar", "vector")]
                    for p in range(2):
                        norm_chain(
                            pvs, OT, q0, p * (QCH // 2), QCH // 2,
                            copy_eng="scalar" if p == 0 else "vector",
                        )
                        while prep_q:
                            prep_q.pop(0)[1]()
                        while prep2_q:
                            prep2_q.pop(0)[1]()
                        for bi in range(2):
                            nb = qh * QCH // 128 + p * 2 + bi
                            for c, fn in proj_quanta(
                                b, OT, nb, engines=engs[bi % 2]
                            ):
                                fn()

            while proj_q:
                proj_q.pop(0)[2]()
    nc.finalize()
    return nc


def _get_bass(with_bias=False):
    key = f"nc{int(with_bias)}"
    if key not in _CACHE:
        _CACHE[key] = _build_bass(with_bias)
    return _CACHE[key]


def _make_in_maps(x, W_qkv, b_qkv, W_out):
    """Shard the full inputs into the 8 per-core input dicts."""
    x = np.asarray(x, dtype=np.float32)
    W_qkv = np.asarray(W_qkv, dtype=np.float32)
    b_qkv = np.asarray(b_qkv, dtype=np.float32)
    W_out = np.asarray(W_out, dtype=np.float32)

    with_bias = bool(np.any(b_qkv))
    # x^T per batch, shared by all cores (+ optional bias chunk rows)
    xtt = x.transpose(0, 2, 1)
    if with_bias:
        aug = np.zeros((B, 128, N), dtype=np.float32)
        aug[:, 0, :] = 1.0
        xtt = np.concatenate([xtt, aug], axis=1)
    xt = np.ascontiguousarray(xtt).astype(BF16)

    in_maps = []
    for c in range(NCORES):
        heads = [HC * c + i for i in range(HC)]
        # W_qkv columns: head h occupies cols [h*3*DP, (h+1)*3*DP) as [q|k|v]
        qcols = [W_qkv[:, h * 3 * DP : h * 3 * DP + DP] for h in heads]
        kcols = [W_qkv[:, h * 3 * DP + DP : h * 3 * DP + 2 * DP] for h in heads]
        vcols = [W_qkv[:, h * 3 * DP + 2 * DP : h * 3 * DP + 3 * DP] for h in heads]
        wsel = np.concatenate(qcols + kcols + vcols, axis=1)  # [D, 3*E]
        if with_bias:
            bq = [b_qkv[h * 3 * DP : h * 3 * DP + DP] for h in heads]
            bk = [b_qkv[h * 3 * DP + DP : h * 3 * DP + 2 * DP] for h in heads]
            bv = [b_qkv[h * 3 * DP + 2 * DP : h * 3 * DP + 3 * DP] for h in heads]
            brow = np.concatenate(bq + bk + bv)  # [3*E]
            baug = np.zeros((128, 3 * E), dtype=np.float32)
            baug[0, :] = brow
            wsel = np.concatenate([wsel, baug], axis=0)
        woutc = np.concatenate(
            [W_out[h * DP : (h + 1) * DP, :] for h in heads], axis=0
        )  # [E, D]
        in_maps.append(
            {
                "xt": xt,
                "wsel": np.ascontiguousarray(wsel).astype(BF16),
                "wout": np.ascontiguousarray(woutc).astype(BF16),
            }
        )
    return in_maps, with_bias


def _run(in_maps, with_bias=False, trace=False):
    from concourse import bass_utils

    nc = _get_bass(with_bias)
    return bass_utils.run_bass_kernel_spmd(
        nc, in_maps, core_ids=list(range(NCORES)), trace=trace
    )


def kernel(x, W_qkv, b_qkv, W_out, b_out, _trace=False):
    in_maps, with_bias = _make_in_maps(x, W_qkv, b_qkv, W_out)
    res = _run(in_maps, with_bias=with_bias, trace=_trace)
    y = np.zeros((B, N, D), dtype=np.float32)
    for r in res.results:
        y += np.asarray(r["y"], dtype=np.float32)
    y += np.asarray(b_out, dtype=np.float32)
    _CACHE["last_result"] = res
    return y
